# revision 3
# baseline (speedup 1.0000x reference)
"""Self-contained Trainium2 Bass kernel for the differentiable A* forward pass.

Contract: kernel(**inputs) takes the FULL unsharded inputs (start_index,
goal_index, cost_maps, nodes, adj, weighted_adj) and returns the full output
(histories, path_maps), matching reference() exactly.

Strategy: the 1024-step t-loop is inherently serial, so the whole search runs
on one NeuronCore and the identical kernel is replicated across all 8 cores
(inputs replicated; core 0's output is used). Per step the kernel does a
two-level argmax over the frontier value vector (4096 nodes as [64,64] SBUF
tiles), an on-chip sparse row fetch (gpsimd.ap_gather of the node's neighbor
triples from an SBUF table + PE one-hot densify), and
one-hot masked state updates -- all register-free (this toolchain's
sequencer SBUF loads are broken on HW). The frontier value `val` is
maintained incrementally and state updates are software-pipelined into the
next step's PE/DMA wait windows via tile_wait_until schedule stamps.
"""
import numpy as np
import concourse.bass as bass
import concourse.tile as tile
from concourse import bacc, mybir, bass_utils, bass_isa

N = 4096
N_P, N_F = 64, 64
K = 80        # neighbor slots (graph max degree 70)
KE = 96       # spare rows 80..96 stay zero; partition starts must be 0/32/64/96
BIGPEN = -131072.0
TMAX = N // 4
N_CORES = 8

_cache = {}


def build_kernel(tmax: int, goal: int, debug: bool = False):
    op = mybir.AluOpType
    f32 = mybir.dt.float32
    u32 = mybir.dt.uint32
    i16 = mybir.dt.int16
    nc = bacc.Bacc("TRN2", target_bir_lowering=False, debug=debug)

    T_in = nc.dram_tensor("T", (K, N * 3), f32, kind="ExternalInput").ap()
    iotaP64_in = nc.dram_tensor("iotaP64", (KE + 1, N_P), f32, kind="ExternalInput").ap()
    iota64t_in = nc.dram_tensor("iota64t", (KE + 1, N_F), f32, kind="ExternalInput").ap()
    g0_in = nc.dram_tensor("g0", (N_P, N_F), f32, kind="ExternalInput").ap()
    open0_in = nc.dram_tensor("open0", (N_P, N_F), f32, kind="ExternalInput").ap()
    h_in = nc.dram_tensor("h", (N_P, N_F), f32, kind="ExternalInput").ap()
    hneg_in = nc.dram_tensor("hneg", (N_P, N_F), f32, kind="ExternalInput").ap()
    iota_in = nc.dram_tensor("iota", (N_P, N_F), f32, kind="ExternalInput").ap()
    iotaP_in = nc.dram_tensor("iotaP", (N_P, 1), f32, kind="ExternalInput").ap()
    iotaP32_in = nc.dram_tensor("iotaP32", (N_P, 1), f32, kind="ExternalInput").ap()
    iota128r_in = nc.dram_tensor("iota128r", (1, N_P), f32, kind="ExternalInput").ap()
    ones_in = nc.dram_tensor("ones", (1, K), f32, kind="ExternalInput").ap()
    ident_in = nc.dram_tensor("ident", (128, 128), f32, kind="ExternalInput").ap()

    hist_out = nc.dram_tensor("hist_out", (N_P, N_F), f32, kind="ExternalOutput").ap()
    log_out = nc.dram_tensor("log_out", (1, TMAX), f32, kind="ExternalOutput").ap()

    with tile.TileContext(nc) as tc:
        with tc.tile_pool(name="state", bufs=1) as sp, \
             tc.tile_pool(name="scratch", bufs=1) as pool, \
             tc.tile_pool(name="ps", bufs=1, space="PSUM") as psp, \
             tc.tile_pool(name="ps2", bufs=2, space="PSUM") as psp2:
            g = sp.tile([N_P, N_F], f32, tag="g")
            hist = sp.tile([N_P, N_F], f32, tag="hist")
            closed = sp.tile([N_P, N_F], f32, tag="closed")
            indlog = sp.tile([1, TMAX], f32, tag="indlog")
            val = sp.tile([N_P, N_F], f32, tag="val")
            ndb = sp.tile([N_P, 1], f32, tag="ndb")
            h_t = sp.tile([N_P, N_F], f32, tag="h_t")
            hneg_t = sp.tile([N_P, N_F], f32, tag="hneg_t")
            iota = sp.tile([N_P, N_F], f32, tag="iota")
            itP = sp.tile([N_P, 1], f32, tag="itP")
            itP32 = sp.tile([N_P, 1], f32, tag="itP32")
            i128r = sp.tile([1, N_P], f32, tag="i128r")
            ones_t = sp.tile([1, K], f32, tag="ones_t")
            ident_t = sp.tile([128, 128], f32, tag="ident_t")
            Tt = sp.tile([K, N * 3], f32, tag="Tt")
            iotaP64 = sp.tile([KE + 1, N_P], f32, tag="iotaP64")
            iota64t = sp.tile([KE + 1, N_F], f32, tag="iota64t")
            nbr = sp.tile([KE + 1, 48], f32, tag="nbr")

            nc.sync.dma_start(g, g0_in)
            nc.sync.dma_start(closed, open0_in)
            nc.sync.dma_start(h_t, h_in)
            nc.sync.dma_start(hneg_t, hneg_in)
            nc.sync.dma_start(iota, iota_in)
            nc.sync.dma_start(itP, iotaP_in)
            nc.sync.dma_start(itP32, iotaP32_in)
            nc.sync.dma_start(i128r, iota128r_in)
            nc.sync.dma_start(ones_t, ones_in)
            nc.sync.dma_start(ident_t, ident_in)
            nc.sync.dma_start(Tt, T_in)
            nc.sync.dma_start(iotaP64, iotaP64_in)
            nc.sync.dma_start(iota64t, iota64t_in)
            nc.vector.memset(nbr, 0.0)
            nc.vector.memset(hist, 0.0)
            nc.vector.memset(ndb, 1.0)
            T3 = Tt.rearrange("p (n c) -> p n c", c=3)

            # preamble: val = ((hist>=closed) * BIGPEN) - (g+h)
            e0 = pool.tile([N_P, N_F], f32, tag="e0")
            gh0 = pool.tile([N_P, N_F], f32, tag="gh0")
            nc.vector.tensor_tensor(out=e0, in0=hist, in1=closed, op=op.is_ge)
            nc.vector.tensor_tensor(out=gh0, in0=g, in1=h_t, op=op.add)
            nc.vector.scalar_tensor_tensor(out=val, in0=e0, scalar=BIGPEN, in1=gh0,
                                           op0=op.mult, op1=op.subtract)

            deferred = []  # (idx, idxm, t1, bc0) from previous step

            STEP_MS = 4.5 * 1e-3
            def stamp(t, off_us):
                return tc.tile_wait_until(t * STEP_MS + off_us * 1e-3)

            for t in range(tmax):
                # ---- phase 1: level-1 argmax ----
                m8 = pool.tile([N_P, 8], f32, tag="m8")
                mi8 = pool.tile([N_P, 8], u32, tag="mi8")
                stamp_ctx = stamp(t, 0.9); stamp_ctx.__enter__()
                nc.vector.max(out=m8, in_=val)
                nc.vector.max_index(out=mi8, in_max=m8, in_values=val)
                t_max = psp.tile([1, N_P], f32, tag="t_max")
                nc.tensor.transpose(t_max, m8[:, 0:1], ident_t[0:N_P, 0:N_P])
                gidxF = pool.tile([N_P, 1], f32, tag="gidxF")
                nc.vector.tensor_scalar(out=gidxF, in0=mi8[:, 0:1], scalar1=1.0,
                                        scalar2=itP32[:, 0:1], op0=op.mult, op1=op.add)

                # ---- phase 3: level-2 argmax + index select ----
                t_gidx = psp.tile([1, N_P], f32, tag="t_gidx")
                nc.tensor.transpose(t_gidx, gidxF, ident_t[0:N_P, 0:N_P])
                gmax8 = pool.tile([1, 8], f32, tag="gmax8")
                nc.vector.max(out=gmax8, in_=t_max)
                eqm = pool.tile([1, N_P], f32, tag="eqm")
                nc.vector.tensor_scalar(out=eqm, in0=t_max[0:1, :],
                                        scalar1=gmax8[0:1, 0:1], scalar2=None,
                                        op0=op.is_equal)
                junkr = pool.tile([1, N_P], f32, tag="junkr")
                indF = pool.tile([1, 1], f32, tag="indF")
                nc.vector.scalar_tensor_tensor(
                    out=junkr, in0=eqm, scalar=1.0, in1=t_gidx[0:1, :],
                    op0=op.mult, op1=op.mult, accum_out=indF)
                nc.scalar.activation(indlog[0:1, t:t + 1], indF,
                                     mybir.ActivationFunctionType.Identity)

                # ---- phase 4: broadcast + gather ----
                bc0 = psp2.tile([K, 1], f32, tag="bc0")
                nc.tensor.matmul(bc0, lhsT=ones_t, rhs=indF, start=True, stop=True)
                idx16 = pool.tile([K, 1], i16, tag="idx16")
                nc.vector.tensor_copy(idx16, bc0[0:K, 0:1])
                nc.gpsimd.ap_gather(
                    out_ap=nbr[0:K, :].rearrange("p (i c) -> p i c", c=3),
                    in_ap=T3, idxs_ap=idx16, channels=K, num_elems=N,
                    d=3, num_idxs=16)
                st = pool.tile([KE + 1, N_P], f32, tag="st")
                dmat = pool.tile([KE + 1, N_F], f32, tag="dmat")
                stamp_ctx.__exit__(None, None, None)
                stamp_ctx = stamp(t, 2.55); stamp_ctx.__enter__()
                nc.vector.tensor_scalar(out=st, in0=iotaP64, scalar1=nbr[:, 0:1],
                                        scalar2=None, op0=op.is_equal)
                nc.vector.tensor_scalar(out=dmat, in0=iota64t, scalar1=nbr[:, 1:2],
                                        scalar2=nbr[:, 2:3], op0=op.is_equal,
                                        op1=op.mult)
                row_t = psp.tile([N_P, N_F], f32, tag="row_t")
                nc.tensor.matmul(row_t, lhsT=st, rhs=dmat, start=True, stop=True)
                stamp_ctx.__exit__(None, None, None)

                # deferred g/closed/parents updates from the previous step at 3.2
                if deferred:
                    d_idx, d_row, d_gb, d_bc0 = deferred.pop()
                    with stamp(t, 2.0):
                        t1 = pool.tile([N_P, N_F], f32, tag="t1")
                        nc.vector.tensor_scalar(out=t1, in0=d_row, scalar1=-1.0,
                                                scalar2=d_gb[:, 0:1],
                                                op0=op.mult, op1=op.add)
                        nc.vector.copy_predicated(g, d_idx, t1)
                        nc.vector.tensor_tensor(out=closed, in0=closed, in1=d_idx, op=op.add)

                # ---- phase 5: flight at 3.6 ----
                stamp_ctx = stamp(t, 1.75); stamp_ctx.__enter__()
                ohg = pool.tile([N_P, N_F], f32, tag="ohg")
                nc.vector.tensor_scalar(out=ohg, in0=iota, scalar1=bc0[0:N_P, 0:1],
                                        scalar2=ndb[:, 0:1], op0=op.is_equal, op1=op.mult)
                # val[ind] += BIGPEN  (masks the selected node out of the frontier)
                nc.vector.scalar_tensor_tensor(out=val, in0=ohg, scalar=2.0 * BIGPEN,
                                               in1=val, op0=op.mult, op1=op.add)
                junkm = pool.tile([N_P, N_F], f32, tag="junkm")
                rs = pool.tile([N_P, 1], f32, tag="rs")
                nc.vector.scalar_tensor_tensor(
                    out=junkm, in0=ohg, scalar=1.0, in1=g,
                    op0=op.mult, op1=op.mult, accum_out=rs)
                avail = pool.tile([N_P, N_F], f32, tag="avail")
                nc.vector.tensor_scalar(out=avail, in0=closed, scalar1=0.0,
                                        scalar2=ndb[:, 0:1], op0=op.is_equal, op1=op.mult)
                nc.vector.tensor_scalar(out=ndb, in0=bc0[0:N_P, 0:1], scalar1=float(goal),
                                        scalar2=ndb[:, 0:1], op0=op.not_equal, op1=op.mult)
                t_rs = psp.tile([1, N_P], f32, tag="t_rs")
                nc.tensor.transpose(t_rs, rs, ident_t[0:N_P, 0:N_P])
                gind_s = pool.tile([1, 1], f32, tag="gind_s")
                nc.vector.reduce_sum(gind_s, t_rs[0:1, :], axis=mybir.AxisListType.X)
                gb = psp.tile([N_P, 1], f32, tag="gb")
                nc.tensor.matmul(gb, lhsT=ones_t[0:1, 0:N_P], rhs=gind_s, start=True, stop=True)
                stamp_ctx.__exit__(None, None, None)

                # ---- phase 6: post (needs row_t) at next-step 0.0 ----
                stamp_ctx = stamp(t + 1, 0.0); stamp_ctx.__enter__()
                vneg = pool.tile([N_P, N_F], f32, tag="vneg")
                nc.vector.scalar_tensor_tensor(out=vneg, in0=row_t, scalar=gb[:, 0:1],
                                               op0=op.subtract, op1=op.add, in1=hneg_t)
                idx = pool.tile([N_P, N_F], mybir.dt.uint8, tag="idx")
                nc.vector.scalar_tensor_tensor(out=idx, in0=row_t, scalar=0.0, in1=avail,
                                               op0=op.not_equal, op1=op.mult)
                nc.vector.copy_predicated(val, idx, vneg)
                stamp_ctx.__exit__(None, None, None)

                deferred.append((idx, row_t, gb, bc0))

            # flush deferred updates of the last step
            d_idx, d_row, d_gb, d_bc0 = deferred.pop()
            t1f = pool.tile([N_P, N_F], f32, tag="t1f")
            nc.vector.tensor_scalar(out=t1f, in0=d_row, scalar1=-1.0,
                                    scalar2=d_gb[:, 0:1],
                                    op0=op.mult, op1=op.add)
            nc.vector.copy_predicated(g, d_idx, t1f)
            nc.vector.tensor_tensor(out=closed, in0=closed, in1=d_idx, op=op.add)

            nc.vector.tensor_scalar(out=hist, in0=val, scalar1=1.5 * BIGPEN,
                                    scalar2=None, op0=op.is_le)
            nc.sync.dma_start(hist_out, hist)
            nc.sync.dma_start(log_out, indlog)

    nc.compile()
    return nc




def make_inputs(wadj_clean: np.ndarray, h: np.ndarray, start: int) -> dict:
    g0 = wadj_clean[start].reshape(N_P, N_F).astype(np.float32)
    open0 = np.zeros((N,), np.float32)
    open0[start] = 1.0
    h2 = h.reshape(N_P, N_F).astype(np.float32)
    T = np.zeros((K, N, 3), np.float32)
    for j in range(N):
        nz = np.nonzero(wadj_clean[j])[0]
        T[:len(nz), j, 0] = nz // N_F
        T[:len(nz), j, 1] = nz % N_F
        T[:len(nz), j, 2] = -wadj_clean[j, nz]
    iotaP64 = np.tile(np.arange(N_P, dtype=np.float32), (KE + 1, 1))
    iotaP64[K:, :] = -1.0
    iota64t = np.tile(np.arange(N_F, dtype=np.float32), (KE + 1, 1))
    iota64t[K:, :] = -1.0
    return {
        "T": T.reshape(K, N * 3),
        "iotaP64": iotaP64,
        "iota64t": iota64t,
        "g0": g0,
        "open0": open0.reshape(N_P, N_F),
        "h": h2,
        "hneg": (-h2),
        "iota": np.arange(N, dtype=np.float32).reshape(N_P, N_F),
        "iotaP": np.arange(N_P, dtype=np.float32).reshape(N_P, 1),
        "iotaP32": (np.arange(N_P, dtype=np.float32) * N_F).reshape(N_P, 1),
        "iota128r": np.arange(N_P, dtype=np.float32).reshape(1, N_P),
        "ones": np.ones((1, K), np.float32),
        "ident": np.eye(128, dtype=np.float32),
    }




def replay_parents(log_f: np.ndarray, wadj_clean: np.ndarray, start: int,
                   goal: int, tmax: int):
    """Rebuild parents from the expansion log (exact integer set replay)."""
    parents = np.full(N, goal, np.int32)
    touched = np.zeros(N, bool)
    touched[start] = True
    ts = tmax - 1
    for t in range(tmax):
        ind = int(round(float(log_f[t])))
        nz = np.nonzero(wadj_clean[ind])[0]
        fresh = nz[~touched[nz]]
        parents[fresh] = ind
        touched[fresh] = True
        if ind == goal:
            ts = t
            break
    return parents, ts


def backtrack(parents_i: np.ndarray, t_final: int, goal: int, tmax: int) -> np.ndarray:
    path = np.zeros((N,), np.int32)
    path[goal] = 1
    loc = parents_i[goal]
    for i in range(tmax):
        if i < t_final:
            path[loc] = 1
            loc = parents_i[loc]
    return path


def kernel(start_index, goal_index, cost_maps, nodes, adj, weighted_adj):
    start = int(np.asarray(start_index))
    goal = int(np.asarray(goal_index))
    h = np.asarray(cost_maps, dtype=np.float32)
    wadj = np.asarray(weighted_adj, dtype=np.float32)

    wadj_clean = np.where(np.isinf(wadj), 0.0, wadj).astype(np.float32)
    np.fill_diagonal(wadj_clean, 0.0)

    key = (TMAX, goal)
    if key not in _cache:
        _cache[key] = build_kernel(TMAX, goal)
    nc = _cache[key]

    kin = make_inputs(wadj_clean, h, start)
    res = bass_utils.run_bass_kernel_spmd(
        nc, [kin] * N_CORES, core_ids=list(range(N_CORES)))
    r0 = res.results[0]
    hist = np.asarray(r0["hist_out"], dtype=np.float32).reshape(N)
    log_f = np.asarray(r0["log_out"], dtype=np.float32).reshape(-1)
    parents, ts = replay_parents(log_f, wadj_clean, start, goal, TMAX)
    path = backtrack(parents, ts, goal, TMAX)
    return hist, path.astype(np.int32)
